# revision 1
# baseline (speedup 1.0000x reference)
"""Causal self-attention (B=2, T=2048, C=1024, H=16, D=64) on 8 trn2 NeuronCores.

Sharding: data-parallel over batch (2) x tensor-parallel over heads (16 -> 4
per core). Core c handles batch c//4 and head-quad c%4 (feature slice of 256).
Each core computes q/k/v projections for its 4 heads, causal attention, and a
partial output projection against its 256-column slice of Wo. The host sums
the 4 partials per batch (the TP all-reduce) and adds bo + Wo @ bv (the value
bias contributes exactly Wo @ bv per token since attention rows sum to 1).

All matmuls run as float32r (fp22) on the PE at full rate. Scores are computed
transposed (S^T[t, q]) so the scalar engine's exp writes P^T directly in the
layout the P@V matmul consumes; softmax runs without max-subtraction (logits
are bounded by |q||k|/8 <= 8) and the denominator comes from an appended
ones-column in the V stationary operand.

The emission order interleaves the projection/transpose "prep" work for
q-chunk qc+1 (and the output projection for qc-1) into the attention t-loops
of q-chunk qc via a deferred work queue, so the PE always has independent
work while the scalar engine grinds through exp.
"""

import numpy as np

B = 2
T = 2048
C = 1024
NH = 16
D = 64
HEADS_PER_CORE = 4
FSLICE = HEADS_PER_CORE * D  # 256 features per core
SCALE = 0.125  # 1/sqrt(64)
N_CORES = 8

TOKB = T // 128  # 16 token blocks
KCH = C // 128  # 8 contraction chunks
QCH = T // 512  # 4 q chunks


import os
ACT_COPIES = os.environ.get("ACT_COPIES", "1") == "1"
F32R_TRANSPOSE = os.environ.get("F32R_TRANSPOSE", "1") == "1"
DIAG_RESTRICT = os.environ.get("DIAG_RESTRICT", "1") == "1"
FAST_RECIP = os.environ.get("FAST_RECIP", "0") == "1"
EXP_PAIR = os.environ.get("EXP_PAIR", "0") == "1"


def _build_nc(repeat=1):
    from collections import deque
    from contextlib import ExitStack

    import concourse.bacc as bacc
    import concourse.mybir as mb
    import concourse.tile as tile
    from concourse.masks import make_identity

    F32 = mb.dt.float32
    F32R = mb.dt.float32r

    nc = bacc.Bacc()
    x_d = nc.dram_tensor("x", [T, C], F32, kind="ExternalInput")
    wq_d = nc.dram_tensor("wq", [FSLICE, C], F32, kind="ExternalInput")
    wk_d = nc.dram_tensor("wk", [FSLICE, C], F32, kind="ExternalInput")
    wv_d = nc.dram_tensor("wv", [FSLICE, C], F32, kind="ExternalInput")
    wo_d = nc.dram_tensor("wo", [C, FSLICE], F32, kind="ExternalInput")
    bq_d = nc.dram_tensor("bq", [1, FSLICE], F32, kind="ExternalInput")
    bk_d = nc.dram_tensor("bk", [1, FSLICE], F32, kind="ExternalInput")
    out_d = nc.dram_tensor("out", [T, C], F32, kind="ExternalOutput")

    with tile.TileContext(nc) as tc, ExitStack() as top:
        # ---- persistent SBUF ----
        perm = top.enter_context(tc.tile_pool(name="perm", bufs=1))
        ident_f32 = perm.tile([128, 128], F32)
        make_identity(nc, ident_f32)
        ident = perm.tile([128, 128], F32R)
        nc.vector.tensor_copy(ident, ident_f32)
        ones_f32 = perm.tile([1, 512], F32)
        nc.vector.memset(ones_f32, 1.0)
        ones512 = perm.tile([1, 512], F32R)
        nc.vector.tensor_copy(ones512, ones_f32)
        bq_sb = perm.tile([1, FSLICE], F32R)
        bk_sb = perm.tile([1, FSLICE], F32R)
        nc.sync.dma_start(out=bq_sb, in_=bq_d[:, :].bitcast(F32R))
        nc.sync.dma_start(out=bk_sb, in_=bk_d[:, :].bitcast(F32R))

        wqT = perm.tile([128, KCH, FSLICE], F32R)  # [c, kc, feat]
        wkT = perm.tile([128, KCH, FSLICE], F32R)
        wvT = perm.tile([128, KCH, FSLICE], F32R)
        woT = perm.tile([128, 2, C], F32R)  # [feat, fc, out]
        qT = [
            [perm.tile([128, 512], F32R, name=f"qT{p}_{qc}") for qc in range(QCH)]
            for p in range(2)
        ]
        kT = [
            [perm.tile([128, 512], F32R, name=f"kT{p}_{qc}") for qc in range(QCH)]
            for p in range(2)
        ]
        v_sb = [
            perm.tile([128, HEADS_PER_CORE, D + 1], F32R, name=f"v{tb}")
            for tb in range(TOKB)
        ]
        oT = [
            [perm.tile([128, 512], F32R, name=f"oT{p}_{qc}") for qc in range(QCH)]
            for p in range(2)
        ]
        ones_col = perm.tile([128, HEADS_PER_CORE], F32)
        nc.vector.memset(ones_col, 1.0)
        for tb in range(TOKB):
            nc.vector.tensor_copy(
                v_sb[tb][:, :, D : D + 1].rearrange("p a c -> p (a c)"), ones_col
            )

        xtp = top.enter_context(tc.tile_pool(name="xtp", bufs=2))
        xTq = {}  # qc -> rotating [c, kc, tok-chunk] tile
        raw = top.enter_context(tc.tile_pool(name="raw", bufs=4))
        scps = top.enter_context(
            tc.tile_pool(name="scps", bufs=(1 if EXP_PAIR else 2), space="PSUM")
        )
        pvps = top.enter_context(tc.tile_pool(name="pvps", bufs=2, space="PSUM"))
        wkps = top.enter_context(tc.tile_pool(name="wkps", bufs=2, space="PSUM"))
        ptp = top.enter_context(tc.tile_pool(name="ptp", bufs=(3 if EXP_PAIR else 4)))
        rcp = top.enter_context(tc.tile_pool(name="rcp", bufs=4))
        outp = top.enter_context(tc.tile_pool(name="outp", bufs=4))

        def copy_out(dst_ap, src_ap, act):
            if act:
                nc.scalar.copy(dst_ap, src_ap)
            else:
                nc.vector.tensor_copy(dst_ap, src_ap)

        def transpose8(dst_ap, srcs, act=False):
            """8 PE transposes into one 2-bank psum tile, one copy out."""
            if F32R_TRANSPOSE:
                tp = scps.tile([128, 8, 128], F32R, name="tp", tag="sc")
                for j, s in enumerate(srcs):
                    nc.tensor.transpose(tp[:, j, :], s, ident)
                copy_out(dst_ap, tp, act)
            else:
                tp = scps.tile([128, 8, 128], F32, name="tp", tag="sc")
                for j, s in enumerate(srcs):
                    nc.tensor.transpose(tp[:, j, :], s.bitcast(F32), ident_f32)
                copy_out(dst_ap, tp.bitcast(F32R), act)

        def x_item(tb, act=False):
            qc = tb // 4
            if tb % 4 == 0:
                xTq[qc] = xtp.tile([128, KCH, 512], F32R, name="xTq")
            x_raw = raw.tile([128, C], F32R, name="x_raw")
            nc.sync.dma_start(
                out=x_raw, in_=x_d[tb * 128 : (tb + 1) * 128, :].bitcast(F32R)
            )
            transpose8(
                xTq[qc][:, :, (tb % 4) * 128 : (tb % 4 + 1) * 128],
                [x_raw[:, kc * 128 : (kc + 1) * 128] for kc in range(KCH)],
                act=act,
            )

        def w_items():
            items = []
            for w_d, wT in ((wq_d, wqT), (wk_d, wkT), (wv_d, wvT)):
                for g in range(2):
                    def it(w_d=w_d, wT=wT, g=g):
                        w_raw = raw.tile([128, C], F32R, name="x_raw")
                        nc.sync.dma_start(
                            out=w_raw,
                            in_=w_d[g * 128 : (g + 1) * 128, :].bitcast(F32R),
                        )
                        transpose8(
                            wT[:, :, g * 128 : (g + 1) * 128],
                            [w_raw[:, kc * 128 : (kc + 1) * 128] for kc in range(KCH)],
                            act=ACT_COPIES,
                        )
                    items.append(it)
            return items

        def wo_items():
            items = []
            wo_raw_box = {}
            def wo_load():
                wo_raw = perm.tile([128, KCH, FSLICE], F32R, name="wo_raw")
                nc.sync.dma_start(
                    out=wo_raw,
                    in_=wo_d.rearrange("(ob p) f -> p ob f", p=128).bitcast(F32R),
                )
                wo_raw_box[0] = wo_raw
            items.append(wo_load)
            for fc in range(2):
                def it2(fc=fc):
                    transpose8(
                        woT[:, fc, :],
                        [
                            wo_raw_box[0][:, ob, fc * 128 : (fc + 1) * 128]
                            for ob in range(KCH)
                        ],
                        act=ACT_COPIES,
                    )
                items.append(it2)
            return items

        def qk_items(qc, act=False, ps=(0, 1)):
            """q/k projection for one q-chunk; one psum reused across items."""
            items = []
            for p in ps:
                for wT, bias_sb, dstT in ((wqT, bq_sb, qT), (wkT, bk_sb, kT)):
                    box = {}
                    def init(wT=wT, bias_sb=bias_sb, p=p, box=box):
                        ps = wkps.tile([128, 512], F32, name="wk_ps")
                        box[0] = ps
                        nc.tensor.matmul(
                            ps,
                            bias_sb[:, p * 128 : (p + 1) * 128],
                            ones512,
                            start=True,
                            stop=False,
                        )
                    items.append(init)
                    for kc in range(KCH):
                        def step(wT=wT, p=p, kc=kc, box=box):
                            nc.tensor.matmul(
                                box[0],
                                wT[:, kc, p * 128 : (p + 1) * 128],
                                xTq[qc][:, kc, :],
                                start=False,
                                stop=(kc == KCH - 1),
                            )
                        items.append(step)
                    def fin(dstT=dstT, p=p, box=box):
                        copy_out(dstT[p][qc], box[0].bitcast(F32R), act)
                    items.append(fin)
            return items

        def v_item(tb, act=False):
            vps = wkps.tile([128, FSLICE], F32, name="wk_ps")
            for kc in range(KCH):
                nc.tensor.matmul(
                    vps,
                    xTq[tb // 4][:, kc, (tb % 4) * 128 : (tb % 4 + 1) * 128],
                    wvT[:, kc, :],
                    start=(kc == 0),
                    stop=(kc == KCH - 1),
                )
            copy_out(
                v_sb[tb][:, :, 0:D],
                vps.rearrange("p (h d) -> p h d", h=HEADS_PER_CORE).bitcast(F32R),
                act,
            )

        def proj_item(qc, tb4):
            tb = qc * 4 + tb4
            pj = scps.tile([128, 1024], F32, name="pj", tag="sc")
            for oc in range(2):
                for p in range(2):
                    nc.tensor.matmul(
                        pj[:, oc * 512 : (oc + 1) * 512],
                        oT[p][qc][:, tb4 * 128 : (tb4 + 1) * 128],
                        woT[:, p, oc * 512 : (oc + 1) * 512],
                        start=(p == 0),
                        stop=(p == 1),
                    )
            ostage = outp.tile([128, C], F32, name="ostage")
            nc.vector.tensor_copy(ostage, pj)
            nc.sync.dma_start(out=out_d[tb * 128 : (tb + 1) * 128, :], in_=ostage)

        def attention_sweep(extra_front=None):
            # ---- attention sweep ----
            for qc in range(QCH):
                ntb = 4 * qc + 4
                items = []
                if qc == 0 and extra_front:
                    items += extra_front
                if qc > 0:
                    items += [
                        (lambda tb4=tb4, q=qc - 1: proj_item(q, tb4)) for tb4 in range(4)
                    ]
                if qc + 1 < QCH:
                    items += prep_items(qc + 1)
                queue = deque(items)
                points = {"left": 2 * ntb}  # injection points in this qc

                def inject():
                    per_pop = max(1, -(-len(queue) // max(1, points["left"])))
                    points["left"] -= 1
                    n = 0
                    while queue and n < per_pop:
                        queue.popleft()()
                        n += 1

                for p in range(2):
                    pv0 = pvps.tile([65, 512], F32, name="pv", tag="pv")
                    pv1 = pvps.tile([65, 512], F32, name="pv", tag="pv")
                    pvs = (pv0, pv1)
                    pts = {}

                    def emit_scores(tb, p=p, qc=qc, pts=pts):
                        e = max(0, tb * 128 - qc * 512) if DIAG_RESTRICT else 0
                        sc = scps.tile([128, 1024], F32, name="sc", tag="sc")
                        for h2 in range(2):
                            nc.tensor.matmul(
                                sc[:, h2 * 512 + e : (h2 + 1) * 512],
                                kT[p][tb // 4][
                                    h2 * 64 : (h2 + 1) * 64,
                                    (tb % 4) * 128 : (tb % 4 + 1) * 128,
                                ],
                                qT[p][qc][h2 * 64 : (h2 + 1) * 64, e:],
                                start=True,
                                stop=True,
                            )
                        pt = ptp.tile([128, 2, 512], F32R, name="pt")
                        nc.scalar.activation(
                            pt[:, :, e:],
                            sc.rearrange("p (h q) -> p h q", h=2)[:, :, e:],
                            mb.ActivationFunctionType.Exp,
                            scale=SCALE,
                        )
                        if tb >= 4 * qc:
                            if DIAG_RESTRICT:
                                # zero the sub-diagonal triangle in the 128-wide
                                # band [e, e+128); cols < e are never read later
                                nc.gpsimd.affine_select(
                                    out=pt[:, :, e : e + 128],
                                    in_=pt[:, :, e : e + 128],
                                    compare_op=mb.AluOpType.is_ge,
                                    fill=0.0,
                                    base=0,
                                    channel_multiplier=-1,
                                    pattern=[[0, 2], [1, 128]],
                                )
                            else:
                                nc.gpsimd.affine_select(
                                    out=pt,
                                    in_=pt,
                                    compare_op=mb.AluOpType.is_ge,
                                    fill=0.0,
                                    base=qc * 512 - tb * 128,
                                    channel_multiplier=-1,
                                    pattern=[[0, 2], [1, 512]],
                                )
                        pts[tb] = pt

                    def emit_scores_pair(tb0, boxes, p=p, qc=qc, pts=pts):
                        e0 = max(0, tb0 * 128 - qc * 512)
                        sc = scps.tile([128, 2, 2, 512], F32, name="scp", tag="sc")
                        for i in range(2):
                            tb = tb0 + i
                            for h2 in range(2):
                                nc.tensor.matmul(
                                    sc[:, i, h2, e0:],
                                    kT[p][tb // 4][
                                        h2 * 64 : (h2 + 1) * 64,
                                        (tb % 4) * 128 : (tb % 4 + 1) * 128,
                                    ],
                                    qT[p][qc][h2 * 64 : (h2 + 1) * 64, e0:],
                                    start=True,
                                    stop=True,
                                )
                        pt = ptp.tile([128, 2, 2, 512], F32R, name="ptp2", tag="pt")
                        nc.scalar.activation(
                            pt[:, :, :, e0:],
                            sc[:, :, :, e0:],
                            mb.ActivationFunctionType.Exp,
                            scale=SCALE,
                        )
                        for i in range(2):
                            tb = tb0 + i
                            e = max(0, tb * 128 - qc * 512)
                            if tb >= 4 * qc:
                                nc.gpsimd.affine_select(
                                    out=pt[:, i, :, e : e + 128],
                                    in_=pt[:, i, :, e : e + 128],
                                    compare_op=mb.AluOpType.is_ge,
                                    fill=0.0,
                                    base=0,
                                    channel_multiplier=-1,
                                    pattern=[[0, 2], [1, 128]],
                                )
                            pts[tb] = pt[:, i]

                    def emit_pv(tb, p=p, qc=qc, ntb=ntb, pvs=pvs, pts=pts):
                        e = max(0, tb * 128 - qc * 512) if DIAG_RESTRICT else 0
                        pt = pts.pop(tb)
                        for h2 in range(2):
                            nc.tensor.matmul(
                                pvs[h2][:, e:],
                                v_sb[tb][:, 2 * p + h2, 0 : D + 1],
                                pt[:, h2, e:],
                                start=(tb == 0),
                                stop=(tb == ntb - 1),
                            )

                    if EXP_PAIR:
                        # scores+exp batched over t-block pairs; PV trails by
                        # one pair; queue work injected before dependent PVs
                        boxes = {}
                        for k in range(ntb // 2):
                            emit_scores_pair(2 * k, boxes)
                            inject()
                            if k > 0:
                                emit_pv(2 * k - 2)
                                emit_pv(2 * k - 1)
                        inject()
                        emit_pv(ntb - 2)
                        emit_pv(ntb - 1)
                        inject()
                    else:
                        # pipeline: PV trails scores by two t-blocks; queue
                        # work injected before each dependent PV
                        emit_scores(0)
                        if ntb > 1:
                            emit_scores(1)
                        for tb in range(2, ntb):
                            emit_scores(tb)
                            inject()
                            emit_pv(tb - 2)
                        inject()
                        emit_pv(ntb - 2)
                        emit_pv(ntb - 1)
                        inject()
                    for h2 in range(2):
                        recip = rcp.tile([1, 512], F32, name="recip")
                        if FAST_RECIP:
                            nc.vector.reciprocal_approx_fast(recip, pvs[h2][64:65, :])
                        else:
                            with nc.allow_low_precision(reason="softmax denom"):
                                nc.vector.reciprocal(recip, pvs[h2][64:65, :])
                        bc_sb = rcp.tile([64, 512], F32, name="bc_sb")
                        nc.gpsimd.partition_broadcast(bc_sb, recip)
                        nc.vector.tensor_tensor(
                            out=oT[p][qc][h2 * 64 : (h2 + 1) * 64, :],
                            in0=pvs[h2][0:64, :],
                            in1=bc_sb,
                            op=mb.AluOpType.mult,
                        )
                while queue:
                    queue.popleft()()
            for tb4 in range(4):
                proj_item(QCH - 1, tb4)


        def body():
            for it in w_items():
                it()
            for it in [
                (lambda tb=tb: x_item(tb, ACT_COPIES)) for tb in range(4)
            ] + qk_items(0, ACT_COPIES, ps=(0,)) + [
                (lambda tb=tb: v_item(tb, ACT_COPIES)) for tb in range(4)
            ]:
                it()
            attention_sweep(
                extra_front=qk_items(0, ACT_COPIES, ps=(1,)) + wo_items()
            )

        def prep_items(qc, act=False):
            items = [
                (lambda tb=tb: x_item(tb, act)) for tb in range(qc * 4, qc * 4 + 4)
            ]
            items += qk_items(qc, act)
            items += [
                (lambda tb=tb: v_item(tb, act)) for tb in range(qc * 4, qc * 4 + 4)
            ]
            return items

        # ---- main body (optionally hardware-looped for timing) ----
        from contextlib import nullcontext

        loop_ctx = tc.For_i(0, repeat, 1) if repeat > 1 else nullcontext()
        with loop_ctx:
            body()

    nc.compile()
    return nc


_NC_CACHE = {}


def _get_nc(repeat=1):
    if repeat not in _NC_CACHE:
        _NC_CACHE[repeat] = _build_nc(repeat)
    return _NC_CACHE[repeat]


def make_in_maps(x, Wq, bq, Wk, bk, Wv, bv, Wo, bo):
    x = np.asarray(x, dtype=np.float32)
    in_maps = []
    for c in range(N_CORES):
        b, p4 = divmod(c, 4)
        fs = slice(p4 * FSLICE, (p4 + 1) * FSLICE)
        in_maps.append(
            {
                "x": np.ascontiguousarray(x[b]),
                "wq": np.ascontiguousarray(np.asarray(Wq)[fs, :]),
                "wk": np.ascontiguousarray(np.asarray(Wk)[fs, :]),
                "wv": np.ascontiguousarray(np.asarray(Wv)[fs, :]),
                "wo": np.ascontiguousarray(np.asarray(Wo)[:, fs]),
                "bq": np.ascontiguousarray(np.asarray(bq)[fs].reshape(1, FSLICE)),
                "bk": np.ascontiguousarray(np.asarray(bk)[fs].reshape(1, FSLICE)),
            }
        )
    return in_maps


def combine_outputs(outs, Wo, bv, bo):
    """outs: list of 8 [T, C] partials. Host-side TP all-reduce + biases."""
    const = np.asarray(bo, dtype=np.float32) + np.asarray(Wo, dtype=np.float32) @ np.asarray(
        bv, dtype=np.float32
    )
    full = np.stack(outs).reshape(B, 4, T, C).sum(axis=1, dtype=np.float32)
    return (full + const[None, None, :]).astype(np.float32)


def kernel(x, Wq, bq, Wk, bk, Wv, bv, Wo, bo):
    from concourse.bass_utils import run_bass_kernel_spmd

    nc = _get_nc()
    in_maps = make_in_maps(x, Wq, bq, Wk, bk, Wv, bv, Wo, bo)
    res = run_bass_kernel_spmd(nc, in_maps, core_ids=list(range(N_CORES)))
    outs = [res.results[c]["out"] for c in range(N_CORES)]
    return combine_outputs(outs, Wo, bv, bo)



# revision 4
# speedup vs baseline: 1.3653x; 1.3653x over previous
"""Causal self-attention (B=2, T=2048, C=1024, H=16, D=64) on 8 trn2 NeuronCores.

Sharding: data-parallel over batch (2) x tensor-parallel over heads (16 -> 4
per core). Core c handles batch c//4 and head-quad c%4 (feature slice of 256).
Each core computes q/k/v projections for its 4 heads, causal attention, and a
partial output projection against its 256-column slice of Wo. The host sums
the 4 partials per batch (the TP all-reduce) and adds bo + Wo @ bv (the value
bias contributes exactly Wo @ bv per token since attention rows sum to 1).

v2: all matmul operands are bf16 (fp32 PSUM accumulation). The host ships
pre-transposed tensors (x^T, Wq^T/Wk^T/Wv^T, Wo^T slices, per-partition
bias vectors), eliminating every PE transpose and the PSUM staging copies
they needed. The q/k bias adds are fused into the PSUM->SBUF copy-out as
DVE tensor_scalar ops instead of PE ones-matmuls. Scores are computed
transposed (S^T[t, q]) so the scalar engine's exp writes P^T (bf16)
directly in the layout the P@V matmul consumes; softmax runs without
max-subtraction (logits are bounded) and the denominator comes from an
appended ones-column in the V stationary operand. Output partials are
stored bf16; the host sums in fp32.

The emission order interleaves the projection "prep" work for q-chunk qc+1
(and the output projection for qc-1) into the attention t-loops of q-chunk
qc via a deferred work queue, so the PE always has independent work while
the scalar engine grinds through exp.
"""

import numpy as np

B = 2
T = 2048
C = 1024
NH = 16
D = 64
HEADS_PER_CORE = 4
FSLICE = HEADS_PER_CORE * D  # 256 features per core
SCALE = 0.125  # 1/sqrt(64)
N_CORES = 8

TOKB = T // 128  # 16 token blocks
KCH = C // 128  # 8 contraction chunks
QCH = T // 512  # 4 q chunks


import os
FAST_RECIP = os.environ.get("FAST_RECIP", "0") == "1"
EXP_PAIR = os.environ.get("EXP_PAIR", "0") == "1"


def _build_nc(repeat=1):
    from collections import deque
    from contextlib import ExitStack

    import concourse.bacc as bacc
    import concourse.mybir as mb
    import concourse.tile as tile

    F32 = mb.dt.float32
    BF16 = mb.dt.bfloat16

    nc = bacc.Bacc()
    xT_d = nc.dram_tensor("xT", [KCH, 128, T], BF16, kind="ExternalInput")
    wq_d = nc.dram_tensor("wqT", [KCH, 128, FSLICE], BF16, kind="ExternalInput")
    wk_d = nc.dram_tensor("wkT", [KCH, 128, FSLICE], BF16, kind="ExternalInput")
    wv_d = nc.dram_tensor("wvT", [KCH, 128, FSLICE], BF16, kind="ExternalInput")
    wo_d = nc.dram_tensor("woT", [2, 128, C], BF16, kind="ExternalInput")
    bq_d = nc.dram_tensor("bqT", [128, 2], F32, kind="ExternalInput")
    bk_d = nc.dram_tensor("bkT", [128, 2], F32, kind="ExternalInput")
    out_d = nc.dram_tensor("out", [T, C], BF16, kind="ExternalOutput")

    with tile.TileContext(nc) as tc, ExitStack() as top:
        # ---- persistent SBUF ----
        perm = top.enter_context(tc.tile_pool(name="perm", bufs=1))
        bqT_sb = perm.tile([128, 2], F32)
        bkT_sb = perm.tile([128, 2], F32)
        wqT = perm.tile([128, KCH, FSLICE], BF16)  # [c, kc, feat]
        wkT = perm.tile([128, KCH, FSLICE], BF16)
        wvT = perm.tile([128, KCH, FSLICE], BF16)
        woT = perm.tile([128, 2, C], BF16)  # [feat, fc, out]
        qT = [
            [perm.tile([128, 512], BF16, name=f"qT{p}_{qc}") for qc in range(QCH)]
            for p in range(2)
        ]
        kT = [
            [perm.tile([128, 512], BF16, name=f"kT{p}_{qc}") for qc in range(QCH)]
            for p in range(2)
        ]
        v_sb = [
            perm.tile([128, HEADS_PER_CORE, D + 1], BF16, name=f"v{tb}")
            for tb in range(TOKB)
        ]
        oT = [
            [perm.tile([128, 512], BF16, name=f"oT{p}_{qc}") for qc in range(QCH)]
            for p in range(2)
        ]
        ones_col = perm.tile([128, HEADS_PER_CORE], BF16)
        nc.vector.memset(ones_col, 1.0)
        for tb in range(TOKB):
            nc.vector.tensor_copy(
                v_sb[tb][:, :, D : D + 1].rearrange("p a c -> p (a c)"), ones_col
            )

        xtp = top.enter_context(tc.tile_pool(name="xtp", bufs=2))
        xTq = {}  # qc -> rotating [c, kc, tok-chunk] tile
        scps = top.enter_context(
            tc.tile_pool(name="scps", bufs=(1 if EXP_PAIR else 2), space="PSUM")
        )
        pvps = top.enter_context(tc.tile_pool(name="pvps", bufs=2, space="PSUM"))
        wkps = top.enter_context(tc.tile_pool(name="wkps", bufs=2, space="PSUM"))
        ptp = top.enter_context(tc.tile_pool(name="ptp", bufs=(3 if EXP_PAIR else 4)))
        rcp = top.enter_context(tc.tile_pool(name="rcp", bufs=4))
        outp = top.enter_context(tc.tile_pool(name="outp", bufs=4))

        def load_items():
            items = []
            for dst, src in (
                (bqT_sb, bq_d[:, :]),
                (bkT_sb, bk_d[:, :]),
                (wqT, wq_d.rearrange("kc p f -> p kc f")),
                (wkT, wk_d.rearrange("kc p f -> p kc f")),
            ):
                items.append(lambda dst=dst, src=src: nc.sync.dma_start(out=dst, in_=src))
            return items

        def load2_items():
            items = [
                lambda: nc.sync.dma_start(
                    out=wvT, in_=wv_d.rearrange("kc p f -> p kc f")
                ),
                lambda: nc.sync.dma_start(
                    out=woT, in_=wo_d.rearrange("fc p o -> p fc o")
                ),
            ]
            return items

        def x_item(qc):
            xTq[qc] = xtp.tile([128, KCH, 512], BF16, name="xTq")
            nc.sync.dma_start(
                out=xTq[qc],
                in_=xT_d[:, :, qc * 512 : (qc + 1) * 512].rearrange(
                    "kc p t -> p kc t"
                ),
            )

        def qk_items(qc, ps=(0, 1)):
            """q/k projection for one q-chunk; bias fused into the copy-out."""
            items = []
            for p in ps:
                for wT, biasT, dstT in ((wqT, bqT_sb, qT), (wkT, bkT_sb, kT)):
                    box = {}
                    for kc in range(KCH):
                        def step(wT=wT, p=p, kc=kc, box=box):
                            if kc == 0:
                                box[0] = wkps.tile([128, 512], F32, name="wk_ps")
                            nc.tensor.matmul(
                                box[0],
                                wT[:, kc, p * 128 : (p + 1) * 128],
                                xTq[qc][:, kc, :],
                                start=(kc == 0),
                                stop=(kc == KCH - 1),
                            )
                        items.append(step)
                    def fin(dstT=dstT, biasT=biasT, p=p, box=box):
                        nc.vector.tensor_scalar(
                            dstT[p][qc],
                            box[0],
                            biasT[:, p : p + 1],
                            None,
                            mb.AluOpType.add,
                        )
                    items.append(fin)
            return items

        def v_item(tb):
            vps = wkps.tile([128, FSLICE], F32, name="wk_ps")
            for kc in range(KCH):
                nc.tensor.matmul(
                    vps,
                    xTq[tb // 4][:, kc, (tb % 4) * 128 : (tb % 4 + 1) * 128],
                    wvT[:, kc, :],
                    start=(kc == 0),
                    stop=(kc == KCH - 1),
                )
            nc.vector.tensor_copy(
                v_sb[tb][:, :, 0:D],
                vps.rearrange("p (h d) -> p h d", h=HEADS_PER_CORE),
            )

        def proj_item(qc, tb4):
            tb = qc * 4 + tb4
            pj = scps.tile([128, 1024], F32, name="pj", tag="sc")
            for oc in range(2):
                for p in range(2):
                    nc.tensor.matmul(
                        pj[:, oc * 512 : (oc + 1) * 512],
                        oT[p][qc][:, tb4 * 128 : (tb4 + 1) * 128],
                        woT[:, p, oc * 512 : (oc + 1) * 512],
                        start=(p == 0),
                        stop=(p == 1),
                    )
            ostage = outp.tile([128, C], BF16, name="ostage")
            nc.vector.tensor_copy(ostage, pj)
            nc.sync.dma_start(out=out_d[tb * 128 : (tb + 1) * 128, :], in_=ostage)

        def attention_sweep(extra_front=None):
            for qc in range(QCH):
                ntb = 4 * qc + 4
                items = []
                if qc == 0 and extra_front:
                    items += extra_front
                if qc > 0:
                    items += [
                        (lambda tb4=tb4, q=qc - 1: proj_item(q, tb4)) for tb4 in range(4)
                    ]
                if qc + 1 < QCH:
                    items += prep_items(qc + 1)
                queue = deque(items)
                points = {"left": 2 * ntb}  # injection points in this qc

                def inject():
                    per_pop = max(1, -(-len(queue) // max(1, points["left"])))
                    points["left"] -= 1
                    n = 0
                    while queue and n < per_pop:
                        queue.popleft()()
                        n += 1

                for p in range(2):
                    pv0 = pvps.tile([65, 512], F32, name="pv", tag="pv")
                    pv1 = pvps.tile([65, 512], F32, name="pv", tag="pv")
                    pvs = (pv0, pv1)
                    pts = {}

                    def emit_scores(tb, p=p, qc=qc, pts=pts):
                        e = max(0, tb * 128 - qc * 512)
                        sc = scps.tile([128, 1024], F32, name="sc", tag="sc")
                        for h2 in range(2):
                            nc.tensor.matmul(
                                sc[:, h2 * 512 + e : (h2 + 1) * 512],
                                kT[p][tb // 4][
                                    h2 * 64 : (h2 + 1) * 64,
                                    (tb % 4) * 128 : (tb % 4 + 1) * 128,
                                ],
                                qT[p][qc][h2 * 64 : (h2 + 1) * 64, e:],
                                start=True,
                                stop=True,
                            )
                        pt = ptp.tile([128, 2, 512], BF16, name="pt")
                        nc.scalar.activation(
                            pt[:, :, e:],
                            sc.rearrange("p (h q) -> p h q", h=2)[:, :, e:],
                            mb.ActivationFunctionType.Exp,
                            scale=SCALE,
                        )
                        if tb >= 4 * qc:
                            # zero the sub-diagonal triangle in the 128-wide
                            # band [e, e+128); cols < e are never read later
                            nc.gpsimd.affine_select(
                                out=pt[:, :, e : e + 128],
                                in_=pt[:, :, e : e + 128],
                                compare_op=mb.AluOpType.is_ge,
                                fill=0.0,
                                base=0,
                                channel_multiplier=-1,
                                pattern=[[0, 2], [1, 128]],
                            )
                        pts[tb] = pt

                    def emit_scores_pair(tb0, p=p, qc=qc, pts=pts):
                        e0 = max(0, tb0 * 128 - qc * 512)
                        sc = scps.tile([128, 2, 2, 512], F32, name="scp", tag="sc")
                        for i in range(2):
                            tb = tb0 + i
                            for h2 in range(2):
                                nc.tensor.matmul(
                                    sc[:, i, h2, e0:],
                                    kT[p][tb // 4][
                                        h2 * 64 : (h2 + 1) * 64,
                                        (tb % 4) * 128 : (tb % 4 + 1) * 128,
                                    ],
                                    qT[p][qc][h2 * 64 : (h2 + 1) * 64, e0:],
                                    start=True,
                                    stop=True,
                                )
                        pt = ptp.tile([128, 2, 2, 512], BF16, name="ptp2", tag="pt")
                        nc.scalar.activation(
                            pt[:, :, :, e0:],
                            sc[:, :, :, e0:],
                            mb.ActivationFunctionType.Exp,
                            scale=SCALE,
                        )
                        for i in range(2):
                            tb = tb0 + i
                            e = max(0, tb * 128 - qc * 512)
                            if tb >= 4 * qc:
                                nc.gpsimd.affine_select(
                                    out=pt[:, i, :, e : e + 128],
                                    in_=pt[:, i, :, e : e + 128],
                                    compare_op=mb.AluOpType.is_ge,
                                    fill=0.0,
                                    base=0,
                                    channel_multiplier=-1,
                                    pattern=[[0, 2], [1, 128]],
                                )
                            pts[tb] = pt[:, i]

                    def emit_pv(tb, p=p, qc=qc, ntb=ntb, pvs=pvs, pts=pts):
                        e = max(0, tb * 128 - qc * 512)
                        pt = pts.pop(tb)
                        for h2 in range(2):
                            nc.tensor.matmul(
                                pvs[h2][:, e:],
                                v_sb[tb][:, 2 * p + h2, 0 : D + 1],
                                pt[:, h2, e:],
                                start=(tb == 0),
                                stop=(tb == ntb - 1),
                            )

                    if EXP_PAIR:
                        for k in range(ntb // 2):
                            emit_scores_pair(2 * k)
                            inject()
                            if k > 0:
                                emit_pv(2 * k - 2)
                                emit_pv(2 * k - 1)
                        inject()
                        emit_pv(ntb - 2)
                        emit_pv(ntb - 1)
                        inject()
                    else:
                        # pipeline: PV trails scores by two t-blocks; queue
                        # work injected before each dependent PV
                        emit_scores(0)
                        if ntb > 1:
                            emit_scores(1)
                        for tb in range(2, ntb):
                            emit_scores(tb)
                            inject()
                            emit_pv(tb - 2)
                        inject()
                        emit_pv(ntb - 2)
                        emit_pv(ntb - 1)
                        inject()
                    for h2 in range(2):
                        recip = rcp.tile([1, 512], F32, name="recip")
                        if FAST_RECIP:
                            nc.vector.reciprocal_approx_fast(recip, pvs[h2][64:65, :])
                        else:
                            with nc.allow_low_precision(reason="softmax denom"):
                                nc.vector.reciprocal(recip, pvs[h2][64:65, :])
                        bc_sb = rcp.tile([64, 512], F32, name="bc_sb")
                        nc.gpsimd.partition_broadcast(bc_sb, recip)
                        nc.vector.tensor_tensor(
                            out=oT[p][qc][h2 * 64 : (h2 + 1) * 64, :],
                            in0=pvs[h2][0:64, :],
                            in1=bc_sb,
                            op=mb.AluOpType.mult,
                        )
                while queue:
                    queue.popleft()()
            for tb4 in range(4):
                proj_item(QCH - 1, tb4)

        def prep_items(qc):
            items = [lambda: x_item(qc)]
            items += qk_items(qc)
            items += [(lambda tb=tb: v_item(tb)) for tb in range(qc * 4, qc * 4 + 4)]
            return items

        def body():
            for it in load_items():
                it()
            x_item(0)
            for it in qk_items(0, ps=(0,)):
                it()
            for it in load2_items():
                it()
            for tb in range(4):
                v_item(tb)
            attention_sweep(extra_front=qk_items(0, ps=(1,)))

        # ---- main body (optionally hardware-looped for timing) ----
        from contextlib import nullcontext

        loop_ctx = tc.For_i(0, repeat, 1) if repeat > 1 else nullcontext()
        with loop_ctx:
            body()

    nc.compile()
    return nc


_NC_CACHE = {}


def _get_nc(repeat=1):
    if repeat not in _NC_CACHE:
        _NC_CACHE[repeat] = _build_nc(repeat)
    return _NC_CACHE[repeat]


def make_in_maps(x, Wq, bq, Wk, bk, Wv, bv, Wo, bo):
    import ml_dtypes

    BF = ml_dtypes.bfloat16
    x = np.asarray(x, dtype=np.float32)
    Wq = np.asarray(Wq, dtype=np.float32)
    Wk = np.asarray(Wk, dtype=np.float32)
    Wv = np.asarray(Wv, dtype=np.float32)
    Wo = np.asarray(Wo, dtype=np.float32)
    bq = np.asarray(bq, dtype=np.float32)
    bk = np.asarray(bk, dtype=np.float32)
    in_maps = []
    xT_by_b = [
        np.ascontiguousarray(x[b].T.astype(BF)).reshape(KCH, 128, T) for b in range(B)
    ]
    for c in range(N_CORES):
        b, p4 = divmod(c, 4)
        fs = slice(p4 * FSLICE, (p4 + 1) * FSLICE)
        in_maps.append(
            {
                "xT": xT_by_b[b],
                "wqT": np.ascontiguousarray(Wq[fs, :].T.astype(BF)).reshape(
                    KCH, 128, FSLICE
                ),
                "wkT": np.ascontiguousarray(Wk[fs, :].T.astype(BF)).reshape(
                    KCH, 128, FSLICE
                ),
                "wvT": np.ascontiguousarray(Wv[fs, :].T.astype(BF)).reshape(
                    KCH, 128, FSLICE
                ),
                "woT": np.ascontiguousarray(Wo[:, fs].T.astype(BF)).reshape(
                    2, 128, C
                ),
                "bqT": np.ascontiguousarray(bq[fs].reshape(2, 128).T),
                "bkT": np.ascontiguousarray(bk[fs].reshape(2, 128).T),
            }
        )
    return in_maps


def combine_outputs(outs, Wo, bv, bo):
    """outs: list of 8 [T, C] bf16 partials. Host-side TP all-reduce + biases."""
    const = np.asarray(bo, dtype=np.float32) + np.asarray(Wo, dtype=np.float32) @ np.asarray(
        bv, dtype=np.float32
    )
    full = (
        np.stack([np.asarray(o, dtype=np.float32) for o in outs])
        .reshape(B, 4, T, C)
        .sum(axis=1, dtype=np.float32)
    )
    return (full + const[None, None, :]).astype(np.float32)


def kernel(x, Wq, bq, Wk, bk, Wv, bv, Wo, bo):
    from concourse.bass_utils import run_bass_kernel_spmd

    nc = _get_nc()
    in_maps = make_in_maps(x, Wq, bq, Wk, bk, Wv, bv, Wo, bo)
    res = run_bass_kernel_spmd(nc, in_maps, core_ids=list(range(N_CORES)))
    outs = [res.results[c]["out"] for c in range(N_CORES)]
    return combine_outputs(outs, Wo, bv, bo)
